# revision 65
# baseline (speedup 1.0000x reference)
"""Multi-head causal attention + RoPE, sharded over 8 TRN2 NeuronCores.

Sharding: core c -> batch b = c//4, head-group g = c%4 (4 of 16 heads).
Each core computes a partial output x[b] @ (its heads' slice); host sums
the 4 group partials per batch.

v2 (all-bf16, gap-free PE schedule):
  - everything bf16 on device (x, weights, rope tiles, exp, v, ct, out);
    fp32 only in PSUM accumulators. rel err ~4.5e-3 (gate 2e-2).
  - xT DMA'd in sq-block order so block-0 projections start ~4us in;
    no dummy warm-up matmuls.
  - scores as K=64 matmuls (no zero-padding: halves LDWEIGHTS traffic,
    rk tiles are [64,512] per head/block).
  - software pipeline: projections+rope for block t+1 are interleaved
    into the attention instruction stream of block t, so the PE always
    has independent matmuls to run while exp (ACT) / rope (DVE) catch
    up. Gap-free PE keeps the HAM clock boosted at 2.4 GHz.
  - softmax normalize without ACT copies: reciprocal of the ones-column
    denominator row straight from PSUM (DVE), K=1 matmul broadcasts the
    reciprocal, one DVE mul writes normalized ctx^T (bf16).
  - out-projection PSUM->SBUF copies alternate ACT/DVE; output DMA'd
    bf16 (host sums the 4 partials per batch in fp32).
"""

import os
import sys

import numpy as np

for _p in ("/opt/trn_rl_repo", "/root/.axon_site/_ro/trn_rl_repo"):
    if os.path.isdir(_p) and _p not in sys.path:
        sys.path.append(_p)

import ml_dtypes  # noqa: E402

import concourse.bass as bass  # noqa: E402
import concourse.mybir as mybir  # noqa: E402
import concourse.tile as tile  # noqa: E402
from concourse import bacc  # noqa: E402
from concourse.bass import ts, ds  # noqa: E402
from concourse.bass_utils import run_bass_kernel_spmd  # noqa: E402

B, S, D = 2, 2048, 1024
HEADS, HD = 16, 64
G = 4                      # head groups == cores per batch element
HPC = HEADS // G           # heads per core
NCOL = HPC * HD            # 256 projection cols per core
KCH = D // 128             # K chunks
MCH = S // 128             # sk chunks
TCH = S // 512             # sq 512-blocks
F32 = mybir.dt.float32
F16 = mybir.dt.float16
F32R = mybir.dt.float32r
BF16 = mybir.dt.bfloat16
AF = mybir.ActivationFunctionType

TRACE = False
TRACE_DIR = None
LAST_EXEC_NS = None
_CACHE = {}


def _build():
    nc = bacc.Bacc("TRN2")
    xT_d = nc.dram_tensor("xT", (D, S), BF16, kind="ExternalInput")
    wqe_d = nc.dram_tensor("wqe", (D, 128), BF16, kind="ExternalInput")
    wqo_d = nc.dram_tensor("wqo", (D, 128), BF16, kind="ExternalInput")
    wke_d = nc.dram_tensor("wke", (D, 128), BF16, kind="ExternalInput")
    wko_d = nc.dram_tensor("wko", (D, 128), BF16, kind="ExternalInput")
    wv_d = nc.dram_tensor("wv", (D, NCOL), BF16, kind="ExternalInput")
    wo_d = nc.dram_tensor("wo", (NCOL, D), BF16, kind="ExternalInput")
    cos_d = nc.dram_tensor("cosT", (128, S), BF16, kind="ExternalInput")
    sin_d = nc.dram_tensor("sinT", (128, S), BF16, kind="ExternalInput")
    tri_d = nc.dram_tensor("tri", (128, 128), BF16, kind="ExternalInput")
    out_d = nc.dram_tensor("out", (S, D), BF16, kind="ExternalOutput")

    mmr = nc.tensor.matmul

    with tile.TileContext(nc) as tc:
        with tc.tile_pool(name="pp", bufs=1) as pp, \
             tc.tile_pool(name="tmp3", bufs=2) as tmp3, \
             tc.tile_pool(name="pexp", bufs=6) as pexp, \
             tc.tile_pool(name="tmp2", bufs=2) as tmp2, \
             tc.tile_pool(name="pA", bufs=1) as pA, \
             tc.tile_pool(name="psP", bufs=2, space="PSUM") as psP, \
             tc.tile_pool(name="psS", bufs=2, space="PSUM") as psS, \
             tc.tile_pool(name="psC", bufs=2, space="PSUM") as psC:
            # roped q/k per (pair, block): 2 heads stacked on partitions
            # (head parity -> rows 0:64 / 64:128), so K=64 score matmuls
            # can slice lhsT and rhs at the same base partition
            rq = [[pp.tile([128, 512], BF16, name=f"rq{p}_{nb}")
                   for nb in range(TCH)] for p in range(2)]
            rk = [[pp.tile([128, 512], BF16, name=f"rk{p}_{nb}")
                   for nb in range(TCH)] for p in range(2)]
            # v (+ones col) per 4-chunk group
            v_grp = [pp.tile([128, 4, HPC, HD + 1], BF16, name=f"vg{g}")
                     for g in range(TCH)]
            tri_sb = pp.tile([128, 128], BF16)
            ones1 = pp.tile([1, 64], BF16)

            w_sb = {}

            # input DMAs spread across engine queues so the start isn't
            # serialized behind one DMA FIFO
            def load_w(nm, d_t, w, eng):
                t_sb = pA.tile([128, KCH, w], BF16, tag=nm, name=nm)
                eng.dma_start(
                    t_sb[:], d_t.rearrange("(ko p) m -> p ko m", p=128))
                w_sb[nm] = t_sb

            # only the sync and gpsimd DMA paths are fast; the ACT path
            # costs 4-10us per transfer regardless of size
            xt = [pA.tile([128, KCH, 512], BF16, name=f"xt{tb}")
                  for tb in range(TCH)]
            nc.sync.dma_start(
                xt[0][:],
                xT_d[:, ts(0, 512)].rearrange("(ko p) m -> p ko m", p=128))
            load_w("wqe", wqe_d, 128, nc.gpsimd)
            load_w("wqo", wqo_d, 128, nc.gpsimd)
            load_w("wv", wv_d, NCOL, nc.sync)
            load_w("wke", wke_d, 128, nc.gpsimd)
            load_w("wko", wko_d, 128, nc.gpsimd)
            cos_sb = pA.tile([128, S], BF16)
            sin_sb = pA.tile([128, S], BF16)
            nc.sync.dma_start(cos_sb[:], cos_d[:])
            nc.sync.dma_start(sin_sb[:], sin_d[:])
            nc.gpsimd.dma_start(tri_sb[:], tri_d[:])
            nc.vector.memset(ones1[:], 1.0)
            nc.sync.dma_start(
                xt[1][:],
                xT_d[:, ts(1, 512)].rearrange("(ko p) m -> p ko m", p=128))
            wo_sb = pA.tile([128, 2, D], BF16)
            nc.gpsimd.dma_start(
                wo_sb[:], wo_d.rearrange("(j p) n -> p j n", p=128))
            for tb in (2, 3):
                nc.sync.dma_start(
                    xt[tb][:],
                    xT_d[:, ts(tb, 512)].rearrange(
                        "(ko p) m -> p ko m", p=128))
            for g in range(TCH):
                nc.vector.memset(v_grp[g][:, :, :, HD], 1.0)
            ct = pA.tile([128, 2, S], BF16)

            def rope_e_half(e_ps, nb, st):
                # cast PSUM->bf16 SBUF first (frees the psP slot fast);
                # all muls then run in the DVE 2x packed mode
                sl = ts(nb, 512)
                eb = tmp3.tile([128, 512], BF16, tag="ropee")
                nc.vector.tensor_copy(eb[:], e_ps[:])
                t1 = tmp3.tile([128, 512], BF16, tag="ropet")
                t3 = tmp3.tile([128, 512], BF16, tag="ropev")
                nc.vector.tensor_mul(t1[:], eb[:], cos_sb[:, sl])
                nc.vector.tensor_mul(t3[:], eb[:], sin_sb[:, sl])
                st["t1"], st["t3"] = t1, t3

            def rope_o_half(o_ps, dt_fn, nb, st):
                sl = ts(nb, 512)
                ob = tmp3.tile([128, 512], BF16, tag="ropeo")
                nc.vector.tensor_copy(ob[:], o_ps[:])
                t2 = tmp3.tile([128, 512], BF16, tag="ropeu")
                t4 = tmp3.tile([128, 512], BF16, tag="ropew")
                nc.vector.tensor_mul(t2[:], ob[:], sin_sb[:, sl])
                nc.vector.tensor_mul(t4[:], ob[:], cos_sb[:, sl])
                t1, t3 = st["t1"], st["t3"]
                for h in range(HPC):
                    d_t, r0 = dt_fn(h)
                    nc.vector.tensor_sub(
                        d_t[r0:r0 + 32, :],
                        t1[32 * h:32 * h + 32, :],
                        t2[32 * h:32 * h + 32, :])
                for h in range(HPC):
                    d_t, r0 = dt_fn(h)
                    nc.vector.tensor_add(
                        d_t[r0 + 32:r0 + 64, :],
                        t3[32 * h:32 * h + 32, :],
                        t4[32 * h:32 * h + 32, :])

            def v_filler(t, m4s):
                # v depends only on xt, so it never waits on the rope
                # chain for psP slots; the PSUM-evacuating copy goes to
                # ACT for early blocks so the DVE queue stays rope-only
                for m4 in m4s:
                    m = 4 * t + m4
                    v_ps = psP.tile([128, NCOL], F32, tag="pq",
                                    name="vps")
                    for kh in range(2):
                        def mm(kh=kh, v_ps=v_ps, m=m):
                            for k in range(4 * kh, 4 * kh + 4):
                                mmr(v_ps[:],
                                    xt[m // 4][:, k, ts(m % 4, 128)],
                                    w_sb["wv"][:, k, :],
                                    start=(k == 0),
                                    stop=(k == KCH - 1))
                        yield mm

                    def vcopy(v_ps=v_ps, t=t, m4=m4):
                        cp = nc.scalar.copy if t <= 2 \
                            else nc.vector.tensor_copy
                        cp(v_grp[t][:, m4, :, 0:HD],
                           v_ps.rearrange("p (h d) -> p h d", h=HPC))
                    yield vcopy

            def qk_filler(t, pool_k=None, tag_k=None):
                """Yield closures: q/k projections + rope for block t."""
                sl = ts(t, 512)

                def qk_group(we, wod, dt_fn, pool, tag):
                    # e-projection fully first: its rope muls start while
                    # the o-projection matmuls still run
                    st = {}
                    e_ps = pool.tile([128, 512], F32, tag=tag,
                                     name="eps")
                    for k in range(KCH):
                        def mm(k=k, e_ps=e_ps, we=we):
                            mmr(e_ps[:], we[:, k, :], xt[t][:, k, :],
                                start=(k == 0), stop=(k == KCH - 1))
                        yield mm
                    yield lambda e_ps=e_ps, st=st: \
                        rope_e_half(e_ps, t, st)
                    o_ps = pool.tile([128, 512], F32, tag=tag,
                                     name="ops")
                    for k in range(KCH):
                        def mm(k=k, o_ps=o_ps, wod=wod):
                            mmr(o_ps[:], wod[:, k, :], xt[t][:, k, :],
                                start=(k == 0), stop=(k == KCH - 1))
                        yield mm
                    yield lambda o_ps=o_ps, dt_fn=dt_fn, st=st: \
                        rope_o_half(o_ps, dt_fn, t, st)

                yield from qk_group(
                    w_sb["wqe"], w_sb["wqo"],
                    lambda h, t=t: (rq[h // 2][t], 64 * (h % 2)),
                    psP, "pq")
                yield from qk_group(
                    w_sb["wke"], w_sb["wko"],
                    lambda h, t=t: (rk[h // 2][t], 64 * (h % 2)),
                    pool_k or psP, tag_k or "pq")

            def outproj_filler(tb, on_dve=False):
                # one [128,1024] PSUM tile + 1 copy + 1 DMA per sq chunk
                for m4 in range(4):
                    m = 4 * tb + m4

                    def step(m=m, m4=m4):
                        o_ps = psS.tile([128, 1024], F32, tag="sc",
                                        name="ops")
                        for j2 in range(2):
                            for j in range(2):
                                mmr(o_ps[:, ts(j2, 512)],
                                    ct[:, j, ts(m, 128)],
                                    wo_sb[:, j, ts(j2, 512)],
                                    start=(j == 0), stop=(j == 1))
                        o_sb = tmp2.tile([128, 1024], BF16, tag="osb")
                        if m4 % 2 == 0 and not on_dve:
                            nc.scalar.copy(o_sb[:], o_ps[:])
                        else:
                            nc.vector.tensor_copy(o_sb[:], o_ps[:])
                        nc.sync.dma_start(out_d[ts(m, 128), :], o_sb[:])
                    yield step

            def attention(t, urgent, relaxed, boundary=()):
                # urgent (next block's q/k proj + rope) is consumed during
                # pair 0 so the DVE rope chain finishes mid-block, never
                # gating the next block's scores; relaxed work (v, outproj)
                # paces through pair 1
                urgent = list(urgent)
                relaxed = list(relaxed)
                fillers = urgent + relaxed
                nf = [0]

                def fill(n):
                    for _ in range(n):
                        if nf[0] < len(fillers):
                            fillers[nf[0]]()
                            nf[0] += 1

                nch = 4 * t + 4
                half_slots = max(1, nch // 2)
                per = 0
                for pair in range(2):
                    # 3 fill points per cb; urgent drains during pair 0
                    if pair == 0:
                        per = -(-len(urgent) // (3 * half_slots))
                    else:
                        for f in boundary:
                            f()
                        per = -(-(len(fillers) - nf[0])
                                // (3 * half_slots))
                    hs = (2 * pair, 2 * pair + 1)
                    ctx_ps = {h: psC.tile([65, 512], F32, tag="ctx",
                                          name=f"ctx{h}")
                              for h in hs}
                    pending = []
                    for cb in range(nch // 2):
                        c0 = 2 * cb
                        scs = {}
                        for half in range(2):
                            c = c0 + half
                            diag = (c // 4 == t)
                            off = 128 * (c % 4) if diag else 0
                            col = slice(512 * half + off, 512 * half + 512)
                            for h in hs:
                                if h not in scs:
                                    scs[h] = psS.tile(
                                        [128, 1024], F32, tag="sc",
                                        name=f"sc{h}")
                                r0 = 64 * (h % 2)
                                mmr(scs[h][:, col],
                                    rk[h // 2][c // 4][
                                        r0:r0 + 64,
                                        ts(c % 4, 128)],
                                    rq[pair][t][r0:r0 + 64,
                                                ds(off, 512 - off)],
                                    start=True, stop=True)
                        fill(per)
                        last_cb = (cb == nch // 2 - 1)
                        nxt = []
                        for h in hs:
                            sc = scs[h]
                            e_sb = pexp.tile([128, 1024], BF16, tag="exp")
                            if last_cb:
                                # diag pair (c0=4t+2): widths 256 and 128
                                nc.scalar.activation(
                                    e_sb[:, 256:512], sc[:, 256:512],
                                    AF.Exp)
                                nc.scalar.activation(
                                    e_sb[:, 896:1024], sc[:, 896:1024],
                                    AF.Exp)
                            else:
                                nc.scalar.activation(e_sb[:], sc[:], AF.Exp)
                            if h == hs[1]:
                                fill(per)
                            for half in range(2):
                                c = c0 + half
                                diag = (c // 4 == t)
                                off = 128 * (c % 4) if diag else 0
                                if diag:
                                    dcol = slice(512 * half + off,
                                                 512 * half + off + 128)
                                    nc.gpsimd.tensor_mul(
                                        e_sb[:, dcol], e_sb[:, dcol],
                                        tri_sb[:])

                                def emit_ctx(h=h, c=c, off=off,
                                             e_sb=e_sb, half=half):
                                    ecol = slice(512 * half + off,
                                                 512 * half + 512)
                                    mmr(ctx_ps[h][:, ds(off, 512 - off)],
                                        v_grp[c // 4][:, c % 4, h, :],
                                        e_sb[:, ecol],
                                        start=(c == 0),
                                        stop=(c == nch - 1))
                                nxt.append(emit_ctx)
                        for f in pending:
                            f()
                        fill(per)
                        pending = nxt
                    for f in pending:
                        f()
                    fill(per)
                    for h in hs:
                        # softmax normalize: denom row -> K=1 matmul
                        # broadcast -> reciprocal -> one mul into ct
                        cpy = nc.vector.tensor_copy \
                            if (t == TCH - 1 and pair == 0) \
                            else nc.scalar.copy
                        cx = tmp2.tile([64, 512], BF16, tag="cx")
                        cpy(cx[:], ctx_ps[h][0:64, :])
                        d_sb = tmp2.tile([1, 512], BF16, tag="dsb")
                        cpy(d_sb[:], ctx_ps[h][64:65, :])
                        rb_ps = psC.tile([64, 512], F32, tag="ctx",
                                         name="rbps")
                        mmr(rb_ps[:], ones1[:], d_sb[:],
                            start=True, stop=True)
                        rec = tmp2.tile([64, 512], F32, tag="rec")
                        nc.vector.reciprocal_approx_fast(
                            rec[:], rb_ps[:])
                        nc.vector.tensor_mul(
                            ct[64 * (h % 2):64 * (h % 2) + 64, h // 2,
                               ts(t, 512)],
                            cx[:], rec[:])
                        fill(per)
                    fill(per)
                fill(len(fillers) - nf[0])

            # t=0 projections up front (K e/o borrow psS slots so the PE
            # isn't gated on rope freeing the psP pair); Q(1) is issued
            # before attention(0) so the PE queue has ready work while
            # rope(0) runs on DVE
            for f in qk_filler(0, psS, "sc"):
                f()
            for f in v_filler(0, (0, 1, 2, 3)):
                f()
            qk1 = list(qk_filler(1))
            for f in qk1[:len(qk1) // 2]:
                f()
            # outproj(t-1) at attention(t)'s pair boundary spreads the
            # PSUM-evacuation copies; block 3 (exp-paced, no projections
            # left) absorbs its own v tail + outproj(2)
            for t in range(TCH):
                urgent, relaxed = [], []
                if t == 0:
                    urgent.extend(qk1[len(qk1) // 2:])
                elif t + 1 < TCH:
                    urgent.extend(qk_filler(t + 1))
                if t + 1 < TCH:
                    relaxed.extend(v_filler(
                        t + 1, (0, 1) if t + 1 == 3 else (0, 1, 2, 3)))
                if t == 3:
                    urgent.extend(v_filler(3, (2, 3)))
                    relaxed.extend(outproj_filler(2, on_dve=True))
                boundary = list(outproj_filler(t - 1)) if t in (1, 2) \
                    else ()
                attention(t, urgent, relaxed, boundary)
            for f in outproj_filler(TCH - 1):
                f()
    nc.compile()
    return nc


def _host_tables():
    half = HD // 2
    inv_freq = (1.0 / (10000.0 ** (np.arange(half, dtype=np.float32) / half)))
    angles = (np.arange(S, dtype=np.float32)[:, None]
              * inv_freq[None, :].astype(np.float32))
    cosT = np.tile(np.cos(angles).T.astype(ml_dtypes.bfloat16), (HPC, 1))
    sinT = np.tile(np.sin(angles).T.astype(ml_dtypes.bfloat16), (HPC, 1))
    i_idx = np.arange(128)[:, None]
    j_idx = np.arange(128)[None, :]
    tri = (j_idx >= i_idx).astype(np.float32)
    return np.ascontiguousarray(cosT), np.ascontiguousarray(sinT), tri


def kernel(x, Wq, Wk, Wv, Wo):
    global LAST_EXEC_NS
    x = np.asarray(x, dtype=np.float32)
    Wq = np.asarray(Wq, dtype=np.float32)
    Wk = np.asarray(Wk, dtype=np.float32)
    Wv = np.asarray(Wv, dtype=np.float32)
    Wo = np.asarray(Wo, dtype=np.float32)

    if "nc" not in _CACHE:
        _CACHE["nc"] = _build()
    nc = _CACHE["nc"]
    cosT, sinT, tri = _host_tables()

    bf = ml_dtypes.bfloat16
    in_maps = []
    for c in range(8):
        b, g = c // 4, c % 4
        cols = slice(g * NCOL, (g + 1) * NCOL)
        wq_g = Wq[:, cols].reshape(D, HPC, HD // 2, 2)
        wk_g = Wk[:, cols].reshape(D, HPC, HD // 2, 2)
        in_maps.append({
            "xT": np.ascontiguousarray(x[b].T).astype(bf),
            "wqe": np.ascontiguousarray(
                wq_g[..., 0].reshape(D, 128) * 0.125).astype(bf),
            "wqo": np.ascontiguousarray(
                wq_g[..., 1].reshape(D, 128) * 0.125).astype(bf),
            "wke": np.ascontiguousarray(
                wk_g[..., 0].reshape(D, 128)).astype(bf),
            "wko": np.ascontiguousarray(
                wk_g[..., 1].reshape(D, 128)).astype(bf),
            "wv": np.ascontiguousarray(Wv[:, cols]).astype(bf),
            "wo": np.ascontiguousarray(Wo[cols, :]).astype(bf),
            "cosT": cosT,
            "sinT": sinT,
            "tri": tri.astype(bf),
        })

    kw = {}
    if TRACE and TRACE_DIR:
        os.makedirs(TRACE_DIR, exist_ok=True)
        kw["tmpdir"] = TRACE_DIR
    res = run_bass_kernel_spmd(nc, in_maps, core_ids=list(range(8)),
                               trace=TRACE, **kw)
    LAST_EXEC_NS = res.exec_time_ns
    parts = [res.results[c]["out"].astype(np.float32) for c in range(8)]
    out = np.empty((B, S, D), dtype=np.float32)
    for b in range(B):
        out[b] = parts[4 * b] + parts[4 * b + 1] + parts[4 * b + 2] \
            + parts[4 * b + 3]
    return out
